# revision 2
# baseline (speedup 1.0000x reference)
"""AtomAttentionEncoder — 8-core SPMD kernel for trn2 (axon-tunneled NeuronCores).

Strategy (per spec sharding_hint): sequence-parallel over the atom (query)
dimension. Each of the 8 cores owns 192 atoms and carries a 192-atom halo on
each side (576-atom local region). The 32x128 block-local attention mask
means block g only attends keys [32g-48, 32g+80), so the pair tensor plm is
only materialized on those windows. The halo lets every layer run with ZERO
inter-core collectives; validity shrinks 2 blocks/side/layer, so each layer
computes only its still-valid region (576/448/320 atoms, 14/10/6 attention
blocks) instead of the full halo every time. All block-window "gathers" are
expressed as 4 static strided slices + concat (no gather ops on device).
Heavy matmuls run in bf16 with fp32 accumulation; layernorm/softmax stay
fp32. The only collective is one psum at the final atom->token aggregation.
"""

import numpy as np
import jax
import jax.numpy as jnp
from functools import partial

B, N_ATOM, N_TOK = 1, 1536, 384
C_ATOM, C_PAIR, C_TOK = 128, 16, 384
C_HID, H, L = 32, 4, 3
HID = 2 * C_ATOM
N_QUERY, N_KEY, INF = 32, 128, 1e9

NCORES = 8
OWN = N_ATOM // NCORES          # 192 atoms owned per core
MARGIN = 192                    # halo per side (6 blocks)
LOC = OWN + 2 * MARGIN          # 576-atom local region
NBLK = LOC // N_QUERY           # 18 local query blocks
PAD = 48                        # window overhang each side
OWN_LO, OWN_HI = MARGIN, MARGIN + OWN
NB_USED = 14                    # blocks 2..15 are the only ones ever attended

BF = jnp.bfloat16


def _mm(x, w):
    """bf16 matmul with fp32 accumulation."""
    return jnp.matmul(x.astype(BF), w.astype(BF),
                      preferred_element_type=jnp.float32)


def _ln(x, gamma=None, beta=None, eps=1e-5):
    mu = jnp.mean(x, axis=-1, keepdims=True)
    var = jnp.mean(jnp.square(x - mu), axis=-1, keepdims=True)
    y = (x - mu) * jax.lax.rsqrt(var + eps)
    if gamma is not None:
        y = y * gamma
    if beta is not None:
        y = y + beta
    return y


def _win_blocks(x, b0, nb):
    """Windows [32g-48, 32g+80) of the leading axis for blocks g in
    [b0, b0+nb), built from 4 static slices (no gather). Requires
    32*b0-48 >= 0 and 32*(b0+nb-1)+80 <= len(x)."""
    lo = 32 * b0 - 48
    parts = [x[lo + 32 * j: lo + 32 * j + 32 * nb].reshape(
        (nb, 32) + x.shape[1:]) for j in range(4)]
    return jnp.concatenate(parts, axis=1)          # [nb, 128, ...]


def _fwd_body(pos, msk, elem, chg, chars, uid, a2t, win_ok, tok_mask,
         W_feats, W_ref_offset, W_inv_sq, W_valid, W_l, W_m,
         W_mlp1, W_mlp2, W_mlp3, W_out_tok,
         attn_ada_gamma_s, attn_ada_Wg, attn_ada_bg, attn_ada_Ws,
         Wq, bq, Wk, Wv, lnz_g, lnz_b, Wb, Wgate, Wo, Wsg, bsg,
         tr_ada_gamma_s, tr_ada_Wg, tr_ada_bg, tr_ada_Ws,
         tr_W1, tr_W2, tr_Wog, tr_bog, tr_Wout):
    # ---- RefAtomFeatureEmbedder on the 576-atom local region ----
    feats = jnp.concatenate(
        [pos, msk[:, None], elem, chg[:, None],
         chars.reshape(LOC, -1), uid[:, None]], axis=-1)
    cl = _mm(feats, W_feats)                                # [576, 128]

    # pair tensor only for blocks 2..15 (the only blocks any layer attends)
    pos_w = _win_blocks(pos, 2, NB_USED)                    # [14,128,3]
    uid_w = _win_blocks(uid, 2, NB_USED)                    # [14,128]
    pos_q = pos[64:64 + 32 * NB_USED].reshape(NB_USED, 32, 3)
    uid_q = uid[64:64 + 32 * NB_USED].reshape(NB_USED, 32)
    d = pos_w[:, None, :, :] - pos_q[:, :, None, :]         # [14,32,128,3]
    v = (uid_w[:, None, :] == uid_q[:, :, None]).astype(jnp.float32)[..., None]
    plm = (d @ W_ref_offset) * v
    inv_sq = 1.0 / (1.0 + jnp.sum(d * d, axis=-1, keepdims=True))
    plm = plm + (inv_sq @ W_inv_sq) * v + (v @ W_valid) * v  # [14,32,128,16]

    crelu = jax.nn.relu(cl)
    cr_l = _mm(crelu, W_l)[64:64 + 32 * NB_USED].reshape(NB_USED, 32, 1, C_PAIR)
    cr_m = _win_blocks(_mm(crelu, W_m), 2, NB_USED)[:, None, :, :]
    plm = plm + cr_l + cr_m
    h = _mm(jax.nn.relu(plm), W_mlp1)
    h = _mm(jax.nn.relu(h), W_mlp2)
    h = _mm(jax.nn.relu(h), W_mlp3)
    plm = plm + h                                           # [14,32,128,16]

    # ---- attention masks (additive, exact -INF semantics of reference) ----
    atom_mask = a2t @ tok_mask                              # [576]
    keymask = _win_blocks(atom_mask, 2, NB_USED) * win_ok   # [14,128] in {0,1}
    addmask = (keymask - 1.0) * INF                         # 0 or -INF
    inv_sqrt = 1.0 / np.sqrt(C_HID)

    ln_s_full = _ln(cl)                                     # hoisted: s is fixed
    a = cl
    for i in range(L):
        Ni = LOC - 128 * i          # valid input atoms this layer
        No = Ni - 128               # valid output atoms
        NBo = No // 32              # attention blocks this layer (14/10/6)
        ob = 2 * i                  # offset into the 14 stored blocks
        in_lo = 64 * i              # offset of this layer's region in [0,576)
        s_in = cl[in_lo:in_lo + Ni]
        ln_s = ln_s_full[in_lo:in_lo + Ni]
        ln_a = _ln(a)                                       # [Ni,128]

        # AttentionPairBias adaLN (an needed on full Ni for k/v)
        sn = ln_s * attn_ada_gamma_s[i]
        an = (jax.nn.sigmoid(_mm(sn, attn_ada_Wg[i]) + attn_ada_bg[i]) * ln_a
              + _mm(sn, attn_ada_Ws[i]))
        an_q = an[64:64 + No]
        q = (_mm(an_q, Wq[i]) + bq[i]).reshape(NBo, 32, H, C_HID)
        kf = _mm(an, Wk[i])
        vf = _mm(an, Wv[i])
        # KV windows for out-block b: rows [16+32b, 144+32b) of the Ni region
        k = jnp.concatenate(
            [kf[16 + 32 * j: 16 + 32 * j + No].reshape(NBo, 32, H, C_HID)
             for j in range(4)], axis=1)                    # [NBo,128,H,32]
        vv = jnp.concatenate(
            [vf[16 + 32 * j: 16 + 32 * j + No].reshape(NBo, 32, H, C_HID)
             for j in range(4)], axis=1)
        zb = _mm(_ln(plm[ob:ob + NBo], lnz_g[i], lnz_b[i]), Wb[i])
        logits = (jnp.einsum('gqhc,gkhc->ghqk', q.astype(BF), k.astype(BF),
                             preferred_element_type=jnp.float32) * inv_sqrt
                  + jnp.moveaxis(zb, -1, 1)
                  + addmask[ob:ob + NBo][:, None, None, :])
        A = jax.nn.softmax(logits, axis=-1)
        o = jnp.einsum('ghqk,gkhc->gqhc', A.astype(BF), vv.astype(BF),
                       preferred_element_type=jnp.float32).reshape(No, H * C_HID)
        g = jax.nn.sigmoid(_mm(an_q, Wgate[i]))
        o = _mm(g * o, Wo[i])
        s_out = s_in[64:64 + No]
        attn_out = jax.nn.sigmoid(_mm(s_out, Wsg[i]) + bsg[i]) * o

        # ConditionedTransitionBlock on pre-attention a (out range only)
        sn_t = (ln_s * tr_ada_gamma_s[i])[64:64 + No]
        tn = (jax.nn.sigmoid(_mm(sn_t, tr_ada_Wg[i]) + tr_ada_bg[i])
              * ln_a[64:64 + No] + _mm(sn_t, tr_ada_Ws[i]))
        hh = jax.nn.silu(_mm(tn, tr_W1[i])) * _mm(tn, tr_W2[i])
        tr_out = (jax.nn.sigmoid(_mm(s_out, tr_Wog[i]) + tr_bog[i])
                  * _mm(hh, tr_Wout[i]))
        a = attn_out + tr_out                               # [No,128]

    # ---- atom -> token mean-aggregation (a is exactly the owned 192 atoms) ----
    al = jax.nn.relu(_mm(a, W_out_tok))                     # [192, 384]
    a2t_own = a2t[OWN_LO:OWN_HI]                            # [192, 384]
    part = a2t_own.T @ al                                   # [384, 384]
    cnt = jnp.sum(a2t_own, axis=0)                          # [384]
    tot = jax.lax.psum(jnp.concatenate([part, cnt[None, :]], axis=0), "x")
    return tot[:N_TOK] / jnp.maximum(tot[N_TOK], 1.0)[:, None]


_fwd = jax.pmap(_fwd_body, axis_name="x", in_axes=(0,) * 8 + (None,) * 35)


def _prep(inputs):
    """Host-side layout: halo shards + constant geometry masks."""
    inp = {k: np.asarray(v) for k, v in inputs.items()}

    # per-core halo shards (layout only: clamped-index slicing)
    starts = np.arange(NCORES) * OWN - MARGIN
    idx = np.clip(starts[:, None] + np.arange(LOC)[None, :], 0, N_ATOM - 1)

    def shard(x):  # x: [1, N_ATOM, ...] -> [8, LOC, ...]
        return x[0][idx]

    # constant geometry mask for blocks 2..15: key global index in [0, N_ATOM)
    gk = (starts[:, None, None]
          + 32 * (2 + np.arange(NB_USED))[None, :, None] - PAD
          + np.arange(N_KEY)[None, None, :])
    win_ok = ((gk >= 0) & (gk < N_ATOM)).astype(np.float32)   # [8,14,128]

    sharded = [shard(inp[n]) for n in
               ('ref_pos', 'ref_mask', 'ref_element', 'ref_charge',
                'ref_atom_name_chars', 'ref_space_uid', 'atom_to_token_index')]
    rep = [inp['token_mask'][0],
           inp['W_feats'], inp['W_ref_offset'], inp['W_inv_sq'],
           inp['W_valid'], inp['W_l'], inp['W_m'], inp['W_mlp1'],
           inp['W_mlp2'], inp['W_mlp3'], inp['W_out_tok'],
           inp['attn_ada_gamma_s'], inp['attn_ada_Wg'], inp['attn_ada_bg'],
           inp['attn_ada_Ws'], inp['Wq'], inp['bq'], inp['Wk'], inp['Wv'],
           inp['lnz_g'], inp['lnz_b'], inp['Wb'], inp['Wgate'], inp['Wo'],
           inp['Wsg'], inp['bsg'], inp['tr_ada_gamma_s'], inp['tr_ada_Wg'],
           inp['tr_ada_bg'], inp['tr_ada_Ws'], inp['tr_W1'], inp['tr_W2'],
           inp['tr_Wog'], inp['tr_bog'], inp['tr_Wout']]

    return sharded + [win_ok] + rep


def kernel(**inputs):
    args = _prep(inputs)
    out = _fwd(*args)
    return np.asarray(out[0])[None].astype(np.float32)       # [1, 384, 384]


def stage(**inputs):
    """Pre-stage shards/weights on the 8 devices (for device-time benchmarks)."""
    args = _prep(inputs)
    devs = jax.devices()[:NCORES]
    staged = []
    for i, a in enumerate(args):
        if i < 8:  # sharded leading-8 args
            staged.append(jax.device_put_sharded(list(a), devs))
        else:
            staged.append(jax.device_put_replicated(a, devs))
    return staged


def run_staged(staged):
    return _fwd_staged(*staged)


@partial(jax.pmap, axis_name="x")
def _fwd_staged(*args):
    return _fwd_body(*args)


# revision 4
# speedup vs baseline: 1.2483x; 1.2483x over previous
"""AtomAttentionEncoder — 8-core SPMD kernel for trn2 (axon-tunneled NeuronCores).

v3: sequence-parallel halo sharding (192 own + 192/side halo, shrinking
validity per layer: 576/448/320 atoms, 14/10/6 attention blocks), slice-based
block windows (no gathers), bf16 matmuls with fp32 accumulation, host-folded
weights (gamma_s into adaLN weights, 1/sqrt(c) into Wq, fused qkv+gate /
ada4 / sg+og / tr12 weight groups -> fewer, wider matmuls), and NO on-device
collective: each core returns its partial token-sum; the unshard step on the
host sums the 8 partials.
"""

import numpy as np
import jax
import jax.numpy as jnp
from functools import partial

B, N_ATOM, N_TOK = 1, 1536, 384
C_ATOM, C_PAIR, C_TOK = 128, 16, 384
C_HID, H, L = 32, 4, 3
HID = 2 * C_ATOM
N_QUERY, N_KEY, INF = 32, 128, 1e9

NCORES = 8
OWN = N_ATOM // NCORES
MARGIN = 192
LOC = OWN + 2 * MARGIN          # 576
PAD = 48
OWN_LO, OWN_HI = MARGIN, MARGIN + OWN
NB_USED = 14                    # blocks 2..15 are the only ones ever attended

BF = jnp.bfloat16


def _mm(x, w):
    return x @ w


def _ln(x, gamma=None, beta=None, eps=1e-5):
    mu = jnp.mean(x, axis=-1, keepdims=True)
    var = jnp.mean(jnp.square(x - mu), axis=-1, keepdims=True)
    y = (x - mu) * jax.lax.rsqrt(var + eps)
    if gamma is not None:
        y = y * gamma
    if beta is not None:
        y = y + beta
    return y


_WIDX = (16 + 32 * np.arange(NB_USED)[:, None]
         + np.arange(N_KEY)[None, :])          # [14,128] rows 16..560


def _win_blocks(x, b0, nb):
    """Windows [32g-48, 32g+80) for blocks g in [b0, b0+nb) = rows
    [16+32b, 144+32b) of x; in-bounds gather (no padding)."""
    return x[_WIDX[:nb]]


def _fwd_body(pos, msk, elem, chg, chars, uid, a2t, win_ok, tok_mask,
              W_feats, W_ref_offset, W_inv_sq, W_valid, W_l, W_m,
              W_mlp1, W_mlp2, W_mlp3, W_out_tok,
              W_ada4, b_ada4, W_qkvg, b_qkvg, lnz_g, lnz_b, Wb, Wo,
              W_sgog, b_sgog, W_tr12, tr_Wout):
    # ---- RefAtomFeatureEmbedder on the 576-atom local region ----
    feats = jnp.concatenate(
        [pos, msk[:, None], elem, chg[:, None],
         chars.reshape(LOC, -1), uid[:, None]], axis=-1)
    cl = _mm(feats, W_feats)                                # [576, 128]

    pos_w = _win_blocks(pos, 2, NB_USED)                    # [14,128,3]
    uid_w = _win_blocks(uid, 2, NB_USED)
    pos_q = pos[64:64 + 32 * NB_USED].reshape(NB_USED, 32, 3)
    uid_q = uid[64:64 + 32 * NB_USED].reshape(NB_USED, 32)
    d = pos_w[:, None, :, :] - pos_q[:, :, None, :]         # [14,32,128,3]
    v = (uid_w[:, None, :] == uid_q[:, :, None]).astype(jnp.float32)[..., None]
    plm = (d @ W_ref_offset) * v
    inv_sq = 1.0 / (1.0 + jnp.sum(d * d, axis=-1, keepdims=True))
    plm = plm + (inv_sq @ W_inv_sq) * v + (v @ W_valid) * v

    crelu = jax.nn.relu(cl)
    cr_l = _mm(crelu, W_l)[64:64 + 32 * NB_USED].reshape(NB_USED, 32, 1, C_PAIR)
    cr_m = _win_blocks(_mm(crelu, W_m), 2, NB_USED)[:, None, :, :]
    plm = plm + cr_l + cr_m
    h = _mm(jax.nn.relu(plm), W_mlp1)
    h = _mm(jax.nn.relu(h), W_mlp2)
    h = _mm(jax.nn.relu(h), W_mlp3)
    plm = plm + h                                           # [14,32,128,16]

    atom_mask = a2t @ tok_mask                              # [576]
    keymask = _win_blocks(atom_mask, 2, NB_USED) * win_ok   # [14,128]
    addmask = (keymask - 1.0) * INF

    ln_s_full = _ln(cl)                                     # s fixed across layers
    a = cl
    for i in range(L):
        Ni = LOC - 128 * i
        No = Ni - 128
        NBo = No // 32
        ob = 2 * i
        in_lo = 64 * i
        s_in = cl[in_lo:in_lo + Ni]
        ln_s = ln_s_full[in_lo:in_lo + Ni]
        ln_a = _ln(a)                                       # [Ni,128]

        # both adaLNs in one matmul (gamma_s folded into W_ada4 on host)
        ada = _mm(ln_s, W_ada4[i]) + b_ada4[i]              # [Ni,512]
        an = jax.nn.sigmoid(ada[:, 0:128]) * ln_a + ada[:, 128:256]
        tn = (jax.nn.sigmoid(ada[64:64 + No, 256:384]) * ln_a[64:64 + No]
              + ada[64:64 + No, 384:512])

        qkvg = _mm(an, W_qkvg[i]) + b_qkvg[i]               # [Ni,512]
        q = qkvg[64:64 + No, 0:128].reshape(NBo, 32, H, C_HID)  # 1/sqrt(c) folded
        kf = qkvg[:, 128:256]
        vf = qkvg[:, 256:384]
        k = kf[_WIDX[:NBo]].reshape(NBo, 128, H, C_HID)
        vv = vf[_WIDX[:NBo]].reshape(NBo, 128, H, C_HID)
        zb = _mm(_ln(plm[ob:ob + NBo], lnz_g[i], lnz_b[i]), Wb[i])
        logits = (jnp.einsum('gqhc,gkhc->ghqk', q, k)
                  + jnp.moveaxis(zb, -1, 1)
                  + addmask[ob:ob + NBo][:, None, None, :])
        A = jax.nn.softmax(logits, axis=-1)
        o = jnp.einsum('ghqk,gkhc->gqhc', A, vv).reshape(No, H * C_HID)
        g = jax.nn.sigmoid(qkvg[64:64 + No, 384:512])
        o = _mm(g * o, Wo[i])

        s_out = s_in[64:64 + No]
        sg = jax.nn.sigmoid(_mm(s_out, W_sgog[i]) + b_sgog[i])  # [No,256]
        attn_out = sg[:, 0:128] * o

        tr12 = _mm(tn, W_tr12[i])                           # [No,512]
        hh = jax.nn.silu(tr12[:, 0:256]) * tr12[:, 256:512]
        tr_out = sg[:, 128:256] * _mm(hh, tr_Wout[i])
        a = attn_out + tr_out                               # [No,128]

    # ---- atom -> token partial aggregation (host sums the 8 partials) ----
    al = jax.nn.relu(_mm(a, W_out_tok))                     # [192, 384]
    a2t_own = a2t[OWN_LO:OWN_HI]
    part = a2t_own.T @ al                                   # [384, 384]
    cnt = jnp.sum(a2t_own, axis=0)                          # [384]
    return jnp.concatenate([part, cnt[None, :]], axis=0)    # [385, 384]


_fwd = jax.pmap(_fwd_body, in_axes=(0,) * 8 + (None,) * 23)


def _prep(inputs):
    """Host-side layout: halo shards, folded/fused weights, geometry masks."""
    inp = {k: np.asarray(v) for k, v in inputs.items()}

    starts = np.arange(NCORES) * OWN - MARGIN
    idx = np.clip(starts[:, None] + np.arange(LOC)[None, :], 0, N_ATOM - 1)

    def shard(x):
        return x[0][idx]

    gk = (starts[:, None, None]
          + 32 * (2 + np.arange(NB_USED))[None, :, None] - PAD
          + np.arange(N_KEY)[None, None, :])
    win_ok = ((gk >= 0) & (gk < N_ATOM)).astype(np.float32)   # [8,14,128]

    # ---- host-side weight folding / fusion (layout only) ----
    ga = inp['attn_ada_gamma_s'][:, :, None]                  # [3,128,1]
    gt = inp['tr_ada_gamma_s'][:, :, None]
    W_ada4 = np.concatenate([ga * inp['attn_ada_Wg'], ga * inp['attn_ada_Ws'],
                             gt * inp['tr_ada_Wg'], gt * inp['tr_ada_Ws']],
                            axis=2)                           # [3,128,512]
    zb128 = np.zeros_like(inp['attn_ada_bg'])
    b_ada4 = np.concatenate([inp['attn_ada_bg'], zb128,
                             inp['tr_ada_bg'], zb128], axis=1)  # [3,512]
    isq = 1.0 / np.sqrt(C_HID)
    W_qkvg = np.concatenate([inp['Wq'] * isq, inp['Wk'], inp['Wv'],
                             inp['Wgate']], axis=2)           # [3,128,512]
    b_qkvg = np.concatenate([inp['bq'] * isq, np.zeros_like(inp['bq']),
                             np.zeros_like(inp['bq']),
                             np.zeros_like(inp['bq'])], axis=1)  # [3,512]
    W_sgog = np.concatenate([inp['Wsg'], inp['tr_Wog']], axis=2)  # [3,128,256]
    b_sgog = np.concatenate([inp['bsg'], inp['tr_bog']], axis=1)  # [3,256]
    W_tr12 = np.concatenate([inp['tr_W1'], inp['tr_W2']], axis=2)  # [3,128,512]

    sharded = [shard(inp[n]) for n in
               ('ref_pos', 'ref_mask', 'ref_element', 'ref_charge',
                'ref_atom_name_chars', 'ref_space_uid', 'atom_to_token_index')]
    rep = [inp['token_mask'][0],
           inp['W_feats'], inp['W_ref_offset'], inp['W_inv_sq'],
           inp['W_valid'], inp['W_l'], inp['W_m'], inp['W_mlp1'],
           inp['W_mlp2'], inp['W_mlp3'], inp['W_out_tok'],
           W_ada4, b_ada4, W_qkvg, b_qkvg,
           inp['lnz_g'], inp['lnz_b'], inp['Wb'], inp['Wo'],
           W_sgog, b_sgog, W_tr12, inp['tr_Wout']]

    return sharded + [win_ok] + rep


def kernel(**inputs):
    args = _prep(inputs)
    parts = np.asarray(_fwd(*args))                          # [8,385,384]
    tot = parts.sum(axis=0)                                  # host unshard-sum
    out = tot[:N_TOK] / np.maximum(tot[N_TOK], 1.0)[:, None]
    return out[None].astype(np.float32)                      # [1, 384, 384]


def stage(**inputs):
    args = _prep(inputs)
    devs = jax.devices()[:NCORES]
    staged = []
    for i, a in enumerate(args):
        if i < 8:
            staged.append(jax.device_put_sharded(list(a), devs))
        else:
            staged.append(jax.device_put_replicated(a, devs))
    return staged


def run_staged(staged):
    return _fwd_staged(*staged)


@partial(jax.pmap)
def _fwd_staged(*args):
    return _fwd_body(*args)


# revision 5
# speedup vs baseline: 1.3718x; 1.0989x over previous
"""AtomAttentionEncoder — 8-core SPMD kernel for trn2 (axon-tunneled NeuronCores).

Known-good baseline (HW 4732586 ns). Restore with: cp kernel_baseline.py kernel.py

Strategy (per spec sharding_hint): sequence-parallel over the atom (query)
dimension. Each of the 8 cores owns 192 atoms and carries a 192-atom halo on
each side (576-atom local region, 18 query blocks of 32). The 32x128
block-local attention mask means block g only attends keys [32g-48, 32g+80),
so the pair tensor plm is only materialized on those windows. The halo lets
every layer run with ZERO inter-core collectives; the only collective is one
psum at the final atom->token aggregation.
"""

import numpy as np
import jax
import jax.numpy as jnp
from functools import partial

B, N_ATOM, N_TOK = 1, 1536, 384
C_ATOM, C_PAIR, C_TOK = 128, 16, 384
C_HID, H, L = 32, 4, 3
HID = 2 * C_ATOM
N_QUERY, N_KEY, INF = 32, 128, 1e9

NCORES = 8
OWN = N_ATOM // NCORES          # 192 atoms owned per core
MARGIN = 192                    # halo per side (6 blocks; >= 64*L)
LOC = OWN + 2 * MARGIN          # 576-atom local region
NBLK = LOC // N_QUERY           # 18 local query blocks
PAD = 48                        # window overhang each side
OWN_LO, OWN_HI = MARGIN, MARGIN + OWN

_WIN_IDX = (32 * np.arange(NBLK)[:, None] + np.arange(N_KEY)[None, :])  # [18,128]


def _ln(x, gamma=None, beta=None, eps=1e-5):
    mu = jnp.mean(x, axis=-1, keepdims=True)
    var = jnp.mean(jnp.square(x - mu), axis=-1, keepdims=True)
    y = (x - mu) * jax.lax.rsqrt(var + eps)
    if gamma is not None:
        y = y * gamma
    if beta is not None:
        y = y + beta
    return y


def _adaln(a, s, gamma_s, Wg, bg, Ws):
    an = _ln(a)
    sn = _ln(s, gamma_s)
    return jax.nn.sigmoid(sn @ Wg + bg) * an + sn @ Ws


def _pad48(x):
    cfg = [(PAD, PAD)] + [(0, 0)] * (x.ndim - 1)
    return jnp.pad(x, cfg)


def _win(x):
    """[LOC(+pad), ...] -> [NBLK, 128, ...] block-local key windows."""
    return _pad48(x)[_WIN_IDX]


def _fwd_body(pos, msk, elem, chg, chars, uid, a2t, win_ok, tok_mask,
         W_feats, W_ref_offset, W_inv_sq, W_valid, W_l, W_m,
         W_mlp1, W_mlp2, W_mlp3, W_out_tok,
         attn_ada_gamma_s, attn_ada_Wg, attn_ada_bg, attn_ada_Ws,
         Wq, bq, Wk, Wv, lnz_g, lnz_b, Wb, Wgate, Wo, Wsg, bsg,
         tr_ada_gamma_s, tr_ada_Wg, tr_ada_bg, tr_ada_Ws,
         tr_W1, tr_W2, tr_Wog, tr_bog, tr_Wout):
    feats = jnp.concatenate(
        [pos, msk[:, None], elem, chg[:, None],
         chars.reshape(LOC, -1), uid[:, None]], axis=-1)
    cl = feats @ W_feats                                    # [LOC, 128]

    pos_w = _win(pos)                                       # [18,128,3]
    uid_w = _win(uid)                                       # [18,128]
    pos_q = pos.reshape(NBLK, N_QUERY, 3)
    uid_q = uid.reshape(NBLK, N_QUERY)
    d = pos_w[:, None, :, :] - pos_q[:, :, None, :]         # [18,32,128,3]
    v = (uid_w[:, None, :] == uid_q[:, :, None]).astype(jnp.float32)[..., None]
    plm = (d @ W_ref_offset) * v
    inv_sq = 1.0 / (1.0 + jnp.sum(d * d, axis=-1, keepdims=True))
    plm = plm + (inv_sq @ W_inv_sq) * v + (v @ W_valid) * v  # [18,32,128,16]

    crelu = jax.nn.relu(cl)
    cr_l = (crelu @ W_l).reshape(NBLK, N_QUERY, 1, C_PAIR)
    cr_m = _win(crelu @ W_m)[:, None, :, :]                  # [18,1,128,16]
    plm = plm + cr_l + cr_m
    h = jax.nn.relu(plm) @ W_mlp1
    h = jax.nn.relu(h) @ W_mlp2
    h = jax.nn.relu(h) @ W_mlp3
    plm = plm + h

    atom_mask = a2t @ tok_mask                               # [LOC]
    keymask = _win(atom_mask) * win_ok                       # [18,128] in {0,1}
    addmask = (keymask - 1.0) * INF                          # 0 or -INF
    inv_sqrt = 1.0 / np.sqrt(C_HID)

    a = s = cl
    for i in range(L):
        an = _adaln(a, s, attn_ada_gamma_s[i], attn_ada_Wg[i],
                    attn_ada_bg[i], attn_ada_Ws[i])
        q = (an @ Wq[i] + bq[i]).reshape(NBLK, N_QUERY, H, C_HID)
        k = _win((an @ Wk[i]).reshape(LOC, H, C_HID))        # [18,128,H,32]
        vv = _win((an @ Wv[i]).reshape(LOC, H, C_HID))
        zb = _ln(plm, lnz_g[i], lnz_b[i]) @ Wb[i]            # [18,32,128,H]
        logits = (jnp.einsum('gqhc,gkhc->ghqk', q, k) * inv_sqrt
                  + jnp.moveaxis(zb, -1, 1)
                  + addmask[:, None, None, :])
        A = jax.nn.softmax(logits, axis=-1)
        o = jnp.einsum('ghqk,gkhc->gqhc', A, vv).reshape(LOC, H * C_HID)
        g = jax.nn.sigmoid(an @ Wgate[i])
        o = (g * o) @ Wo[i]
        attn_out = jax.nn.sigmoid(s @ Wsg[i] + bsg[i]) * o
        tn = _adaln(a, s, tr_ada_gamma_s[i], tr_ada_Wg[i],
                    tr_ada_bg[i], tr_ada_Ws[i])
        hh = jax.nn.silu(tn @ tr_W1[i]) * (tn @ tr_W2[i])
        tr_out = jax.nn.sigmoid(s @ tr_Wog[i] + tr_bog[i]) * (hh @ tr_Wout[i])
        a = attn_out + tr_out

    al = jax.nn.relu(a[OWN_LO:OWN_HI] @ W_out_tok)           # [192, 384]
    a2t_own = a2t[OWN_LO:OWN_HI]                             # [192, 384]
    part = a2t_own.T @ al                                    # [384, 384]
    cnt = jnp.sum(a2t_own, axis=0)                           # [384]
    tot = jax.lax.psum(jnp.concatenate([part, cnt[None, :]], axis=0), "x")
    return tot[:N_TOK] / jnp.maximum(tot[N_TOK], 1.0)[:, None]


_fwd = jax.pmap(_fwd_body, axis_name="x", in_axes=(0,) * 8 + (None,) * 35)


def _prep(inputs):
    inp = {k: np.asarray(v) for k, v in inputs.items()}

    starts = np.arange(NCORES) * OWN - MARGIN
    idx = np.clip(starts[:, None] + np.arange(LOC)[None, :], 0, N_ATOM - 1)

    def shard(x):  # x: [1, N_ATOM, ...] -> [8, LOC, ...]
        return x[0][idx]

    gk = (starts[:, None, None] + 32 * np.arange(NBLK)[None, :, None] - PAD
          + np.arange(N_KEY)[None, None, :])
    win_ok = ((gk >= 0) & (gk < N_ATOM)).astype(np.float32)   # [8,18,128]

    sharded = [shard(inp[n]) for n in
               ('ref_pos', 'ref_mask', 'ref_element', 'ref_charge',
                'ref_atom_name_chars', 'ref_space_uid', 'atom_to_token_index')]
    rep = [inp['token_mask'][0],
           inp['W_feats'], inp['W_ref_offset'], inp['W_inv_sq'],
           inp['W_valid'], inp['W_l'], inp['W_m'], inp['W_mlp1'],
           inp['W_mlp2'], inp['W_mlp3'], inp['W_out_tok'],
           inp['attn_ada_gamma_s'], inp['attn_ada_Wg'], inp['attn_ada_bg'],
           inp['attn_ada_Ws'], inp['Wq'], inp['bq'], inp['Wk'], inp['Wv'],
           inp['lnz_g'], inp['lnz_b'], inp['Wb'], inp['Wgate'], inp['Wo'],
           inp['Wsg'], inp['bsg'], inp['tr_ada_gamma_s'], inp['tr_ada_Wg'],
           inp['tr_ada_bg'], inp['tr_ada_Ws'], inp['tr_W1'], inp['tr_W2'],
           inp['tr_Wog'], inp['tr_bog'], inp['tr_Wout']]

    return sharded + [win_ok] + rep


def kernel(**inputs):
    args = _prep(inputs)
    out = _fwd(*args)
    return np.asarray(out[0])[None].astype(np.float32)       # [1, 384, 384]


def stage(**inputs):
    args = _prep(inputs)
    devs = jax.devices()[:NCORES]
    staged = []
    for i, a in enumerate(args):
        if i < 8:
            staged.append(jax.device_put_sharded(list(a), devs))
        else:
            staged.append(jax.device_put_replicated(a, devs))
    return staged


def run_staged(staged):
    return _fwd_staged(*staged)


@partial(jax.pmap, axis_name="x")
def _fwd_staged(*args):
    return _fwd_body(*args)
